# revision 6
# baseline (speedup 1.0000x reference)
"""Decoder block (8-head causal attention + FFN + 2x layernorm) on 8 trn2 cores.

Problem: x (4, 2048, 512) fp32; per-head Wq/Wk/Wv (8, 512, 64); Wo (512, 512);
FFN 512->2048->512; causal mask; two post-residual layernorms.

Sharding (uniform SPMD program, 8 cores): core c -> (batch n = c//2,
head-half s = c%2). Each core computes Q/K/V for its 4 heads over the full
2048-token sequence of its batch, causal attention, and its partial Wo
projection (contraction over its 256 attention channels). A pairwise
ReduceScatter sums the two partial Wo outputs and hands each core 1024 rows.
Each core then does residual+LN1, the full FFN (512->2048->512) and
residual+LN2 for its 1024 rows. Host reassembles (4, 2048, 512).

All matmuls run as float32r (TF32-like, 1 cycle/row at N>=256) with fp32 PSUM
accumulation. Causality is exploited: fully-masked key blocks are skipped,
diagonal blocks use one static 128x128 additive triangle mask; softmax runs
without max-subtraction (scores are O(10), exp is safe in fp32) and the
denominator comes for free from an appended ones-column in V (M=65 matmul).
"""

import sys

sys.path.insert(0, "/opt/trn_rl_repo")

import numpy as np

import concourse.bacc as bacc
import concourse.bass as bass
import concourse.mybir as mybir
import concourse.tile as tile
from concourse import bass_utils, masks

F32 = mybir.dt.float32
F32R = mybir.dt.float32r
AF = mybir.ActivationFunctionType

N, K, D, H, F = 4, 2048, 512, 8, 2048
Dh = D // H  # 64
HH = H // 2  # 4 local heads per core
E = HH * Dh  # 256 local attention channels
EPS = 1e-10
N_CORES = 8
OWN = K // 2  # 1024 rows per core after ReduceScatter

_CACHE = {}


def _build():
    nc = bacc.Bacc("TRN2", target_bir_lowering=False, debug=False,
                   num_devices=N_CORES)

    xn_d = nc.dram_tensor("xn", [K, D], F32, kind="ExternalInput")
    xres_d = nc.dram_tensor("xres", [OWN, D], F32, kind="ExternalInput")
    wq_d = nc.dram_tensor("wq", [D, E], F32, kind="ExternalInput")
    wk_d = nc.dram_tensor("wk", [D, E], F32, kind="ExternalInput")
    wv_d = nc.dram_tensor("wv", [D, E], F32, kind="ExternalInput")
    bq_d = nc.dram_tensor("bq2", [1, E], F32, kind="ExternalInput")
    bk_d = nc.dram_tensor("bk2", [1, E], F32, kind="ExternalInput")
    bv_d = nc.dram_tensor("bv2", [1, E], F32, kind="ExternalInput")
    wo_d = nc.dram_tensor("wo", [E, D], F32, kind="ExternalInput")
    bo_d = nc.dram_tensor("bo2", [1, D], F32, kind="ExternalInput")
    w1_d = nc.dram_tensor("w1", [D, F], F32, kind="ExternalInput")
    b1_d = nc.dram_tensor("b12", [1, F], F32, kind="ExternalInput")
    w2_d = nc.dram_tensor("w2", [F, D], F32, kind="ExternalInput")
    b2_d = nc.dram_tensor("b22", [1, D], F32, kind="ExternalInput")
    g1_d = nc.dram_tensor("g1", [1, D], F32, kind="ExternalInput")
    be1_d = nc.dram_tensor("be1", [1, D], F32, kind="ExternalInput")
    g2_d = nc.dram_tensor("g2", [1, D], F32, kind="ExternalInput")
    be2_d = nc.dram_tensor("be2", [1, D], F32, kind="ExternalInput")
    out_d = nc.dram_tensor("out", [OWN, D], F32, kind="ExternalOutput")

    def bcast(dram, n):
        # [1, n] DRAM row broadcast to [128, n]
        return bass.AP(tensor=dram, offset=0, ap=[[0, 128], [1, n]])

    with tile.TileContext(nc) as tc:
        import contextlib
        stack = contextlib.ExitStack()
        with stack:
            singles = stack.enter_context(tc.tile_pool(name="singles", bufs=1))
            dram = stack.enter_context(
                tc.tile_pool(name="dram", bufs=1, space="DRAM"))

            # ---- static tiles ----
            ident = singles.tile([128, 128], F32)
            masks.make_identity(nc, ident[:])
            tri = singles.tile([128, 128], F32)
            nc.gpsimd.memset(tri, 0.0)
            # keep 0.0 where q - k >= 0, else -1e9 (partition = key, free = query)
            nc.gpsimd.affine_select(
                out=tri, in_=tri, compare_op=mybir.AluOpType.is_ge,
                fill=-1e9, base=0, pattern=[[1, 128]], channel_multiplier=-1)
            ones_f32 = singles.tile([128, 512], F32)
            nc.vector.memset(ones_f32, 1.0)
            onesr = singles.tile([1, 512], F32R)
            nc.vector.tensor_copy(out=onesr[:], in_=ones_f32[0:1, :])
            ones128r = singles.tile([1, 128], F32R)
            nc.vector.tensor_copy(out=ones128r[:], in_=ones_f32[0:1, 0:128])
            ones4 = singles.tile([128, HH, 1], F32)
            nc.vector.memset(ones4, 1.0)
            eps_t = singles.tile([128, 1], F32)
            nc.vector.memset(eps_t, EPS)

            # gains/biases broadcast to 128 partitions
            g1_bc = singles.tile([128, D], F32)
            nc.gpsimd.dma_start(out=g1_bc, in_=bcast(g1_d, D))
            be1_bc = singles.tile([128, D], F32)
            nc.gpsimd.dma_start(out=be1_bc, in_=bcast(be1_d, D))
            g2_bc = singles.tile([128, D], F32)
            nc.gpsimd.dma_start(out=g2_bc, in_=bcast(g2_d, D))
            be2_bc = singles.tile([128, D], F32)
            nc.gpsimd.dma_start(out=be2_bc, in_=bcast(be2_d, D))

            # small bias rows (f32r)
            bq_sb = singles.tile([1, E], F32R)
            nc.sync.dma_start(out=bq_sb, in_=bq_d[:].bitcast(F32R))
            bk_sb = singles.tile([1, E], F32R)
            nc.sync.dma_start(out=bk_sb, in_=bk_d[:].bitcast(F32R))
            bv_sb = singles.tile([1, E], F32R)
            nc.sync.dma_start(out=bv_sb, in_=bv_d[:].bitcast(F32R))
            bo_sb = singles.tile([1, D], F32R)
            nc.sync.dma_start(out=bo_sb, in_=bo_d[:].bitcast(F32R))
            b1_sb = singles.tile([1, F], F32R)
            nc.sync.dma_start(out=b1_sb, in_=b1_d[:].bitcast(F32R))
            b2_sb = singles.tile([1, D], F32R)
            nc.sync.dma_start(out=b2_sb, in_=b2_d[:].bitcast(F32R))

            # persistent activation tensors
            kt_pool = stack.enter_context(tc.tile_pool(name="kt", bufs=1))
            qt_pool = stack.enter_context(tc.tile_pool(name="qt", bufs=1))
            va_pool = stack.enter_context(tc.tile_pool(name="va", bufs=1))
            ac_pool = stack.enter_context(tc.tile_pool(name="ac", bufs=1))
            kT = [kt_pool.tile([128, K], F32R, name=f"kT{i}") for i in range(2)]
            qT = [qt_pool.tile([128, K], F32R, name=f"qT{i}") for i in range(2)]
            va = [va_pool.tile([128, HH, Dh + 1], F32R, name=f"va{i}")
                  for i in range(K // 128)]
            ac = [ac_pool.tile([128, K], F32R, name=f"ac{i}") for i in range(2)]

            # ---------------- phase 1: xT + projections ----------------
            with tc.tile_pool(name="pw", bufs=1) as pw, \
                 tc.tile_pool(name="xp", bufs=4) as xp, \
                 tc.tile_pool(name="xt", bufs=1) as xtp, \
                 tc.tile_pool(name="ps_tr1", bufs=2, space="PSUM") as ps_tr, \
                 tc.tile_pool(name="ps_proj", bufs=3, space="PSUM") as ps_proj:
                wq_sb = [pw.tile([128, E], F32R, name=f"wq{i}") for i in range(4)]
                wk_sb = [pw.tile([128, E], F32R, name=f"wk{i}") for i in range(4)]
                wv_sb = [pw.tile([128, E], F32R, name=f"wv{i}") for i in range(4)]
                for dc in range(4):
                    nc.sync.dma_start(
                        out=wq_sb[dc],
                        in_=wq_d[dc * 128:(dc + 1) * 128, :].bitcast(F32R))
                    nc.sync.dma_start(
                        out=wk_sb[dc],
                        in_=wk_d[dc * 128:(dc + 1) * 128, :].bitcast(F32R))
                    nc.sync.dma_start(
                        out=wv_sb[dc],
                        in_=wv_d[dc * 128:(dc + 1) * 128, :].bitcast(F32R))

                xT = [xtp.tile([128, K], F32R, name=f"xT{i}") for i in range(4)]
                for kt_i in range(K // 128):
                    xrow = xp.tile([128, D], F32, name="xrow")
                    nc.sync.dma_start(
                        out=xrow, in_=xn_d[kt_i * 128:(kt_i + 1) * 128, :])
                    for dc in range(4):
                        trp = ps_tr.tile([128, 128], F32, name="trp")
                        nc.tensor.transpose(
                            trp[:], xrow[:, dc * 128:(dc + 1) * 128], ident[:])
                        nc.scalar.copy(
                            out=xT[dc][:, kt_i * 128:(kt_i + 1) * 128],
                            in_=trp[:])

                # kT / qT: per head-pair hp, 512-wide key/query block kb
                for w_sb, b_sb, dstT in ((wk_sb, bk_sb, kT), (wq_sb, bq_sb, qT)):
                    for hp in range(2):
                        for kb in range(4):
                            pp = ps_proj.tile([128, 512], F32, name="pp")
                            for dc in range(4):
                                nc.tensor.matmul(
                                    pp[:],
                                    w_sb[dc][:, hp * 128:(hp + 1) * 128],
                                    xT[dc][:, kb * 512:(kb + 1) * 512],
                                    start=(dc == 0), stop=False)
                            nc.tensor.matmul(
                                pp[:], b_sb[0:1, hp * 128:(hp + 1) * 128],
                                onesr[:], start=False, stop=True)
                            nc.scalar.copy(
                                out=dstT[hp][:, kb * 512:(kb + 1) * 512],
                                in_=pp[:])

                # v rows (all 4 heads at once), augmented with ones column
                for kt_i in range(K // 128):
                    vp = ps_proj.tile([128, E], F32, name="vp")
                    for dc in range(4):
                        nc.tensor.matmul(
                            vp[:],
                            xT[dc][:, kt_i * 128:(kt_i + 1) * 128],
                            wv_sb[dc][:], start=(dc == 0), stop=False)
                    nc.tensor.matmul(vp[:], ones128r[:], bv_sb[:],
                                     start=False, stop=True)
                    nc.scalar.copy(
                        out=va[kt_i][:, :, 0:Dh],
                        in_=vp[:].rearrange("p (h e) -> p h e", h=HH))
                    nc.vector.tensor_copy(out=va[kt_i][:, :, Dh:Dh + 1],
                                          in_=ones4[:])

            # ---------------- phase 2: causal attention ----------------
            with tc.tile_pool(name="ps_s", bufs=3, space="PSUM") as ps_s, \
                 tc.tile_pool(name="ps_att", bufs=2, space="PSUM") as ps_att, \
                 tc.tile_pool(name="ps_bc", bufs=2, space="PSUM") as ps_bc, \
                 tc.tile_pool(name="expp", bufs=4) as expp, \
                 tc.tile_pool(name="bcp", bufs=3) as bcp:
                for h in range(HH):
                    hp, h2 = divmod(h, 2)
                    erow = slice(h2 * 64, h2 * 64 + 64)
                    for qb in range(4):
                        qs = qb * 512
                        att_ps = ps_att.tile([65, 512], F32, name="att_ps")
                        n_mm = 0
                        for kb in range(4 * qb):  # full key blocks
                            s_ps = ps_s.tile([128, 512], F32, name="s_ps")
                            nc.tensor.matmul(
                                s_ps[:],
                                kT[hp][erow, kb * 128:(kb + 1) * 128],
                                qT[hp][erow, qs:qs + 512],
                                start=True, stop=True)
                            expT = expp.tile([128, 512], F32R, name="expT")
                            nc.scalar.activation(out=expT[:], in_=s_ps[:],
                                                 func=AF.Exp, scale=0.125)
                            nc.tensor.matmul(
                                att_ps[:], va[kb][:, h, :], expT[:],
                                start=(n_mm == 0), stop=False)
                            n_mm += 1
                        for m in range(4):  # diagonal key blocks
                            kb = 4 * qb + m
                            lo = m * 128
                            s_ps = ps_s.tile([128, 512], F32, name="s_ps")
                            if lo + 128 < 512:
                                nc.tensor.matmul(
                                    s_ps[:, lo + 128:512],
                                    kT[hp][erow, kb * 128:(kb + 1) * 128],
                                    qT[hp][erow, qs + lo + 128:qs + 512],
                                    start=True, stop=True)
                            nc.tensor.matmul(
                                s_ps[:, lo:lo + 128],
                                kT[hp][erow, kb * 128:(kb + 1) * 128],
                                qT[hp][erow, qs + lo:qs + lo + 128],
                                start=True, stop=True)
                            nc.vector.tensor_add(out=s_ps[:, lo:lo + 128],
                                                 in0=s_ps[:, lo:lo + 128],
                                                 in1=tri[:])
                            expT = expp.tile([128, 512], F32R, name="expT")
                            nc.scalar.activation(out=expT[:, lo:512],
                                                 in_=s_ps[:, lo:512],
                                                 func=AF.Exp, scale=0.125)
                            nc.tensor.matmul(
                                att_ps[:, lo:512], va[kb][:, h, :],
                                expT[:, lo:512],
                                start=(n_mm == 0), stop=(m == 3))
                            n_mm += 1
                        # softmax normalize via ones-row denominator
                        recip = bcp.tile([1, 512], F32R, name="recip")
                        with nc.allow_low_precision(reason="f32r recip"):
                            nc.vector.reciprocal(out=recip[:],
                                                 in_=att_ps[64:65, :])
                        bc_ps = ps_bc.tile([64, 512], F32, name="bc_ps")
                        nc.tensor.matmul(bc_ps[:], onesr[0:1, 0:64], recip[:],
                                         start=True, stop=True)
                        bc_sb = bcp.tile([64, 512], F32, name="bc_sb")
                        nc.scalar.copy(out=bc_sb[:], in_=bc_ps[:])
                        with nc.allow_low_precision(reason="f32r attn out"):
                            nc.vector.tensor_mul(
                                out=ac[hp][erow, qs:qs + 512],
                                in0=att_ps[0:64, :], in1=bc_sb[:])

            # ---------------- phase 3: Wo partial + ReduceScatter ----------
            rs_in = dram.tile([K, D], F32, name="rs_in")
            rs_out = dram.tile([OWN, D], F32, name="rs_out")
            with tc.tile_pool(name="wop", bufs=1) as wop, \
                 tc.tile_pool(name="ps_o", bufs=3, space="PSUM") as ps_o, \
                 tc.tile_pool(name="op", bufs=3) as op:
                wo_sb = [wop.tile([128, D], F32R, name=f"wo{i}") for i in range(2)]
                for hp in range(2):
                    nc.sync.dma_start(
                        out=wo_sb[hp],
                        in_=wo_d[hp * 128:(hp + 1) * 128, :].bitcast(F32R))
                for qt in range(K // 128):
                    o_ps = ps_o.tile([128, D], F32, name="o_ps")
                    for hp in range(2):
                        nc.tensor.matmul(
                            o_ps[:], ac[hp][:, qt * 128:(qt + 1) * 128],
                            wo_sb[hp][:], start=(hp == 0), stop=False)
                    nc.tensor.matmul(o_ps[:], ones128r[:], bo_sb[:],
                                     start=False, stop=True)
                    o_sb = op.tile([128, D], F32, name="o_sb")
                    nc.scalar.copy(out=o_sb[:], in_=o_ps[:])
                    nc.sync.dma_start(
                        out=rs_in[qt * 128:(qt + 1) * 128, :], in_=o_sb[:])
            nc.gpsimd.collective_compute(
                "ReduceScatter", mybir.AluOpType.add,
                replica_groups=[[0, 1], [2, 3], [4, 5], [6, 7]],
                ins=[rs_in[:]], outs=[rs_out[:]])

            # ---------------- phase 4: residual + LN1 + h1T ----------------
            h1_pool = stack.enter_context(tc.tile_pool(name="h1", bufs=1))
            h1t_pool = stack.enter_context(tc.tile_pool(name="h1t", bufs=1))
            h1 = [h1_pool.tile([128, D], F32, name=f"h1_{i}")
                  for i in range(OWN // 128)]
            h1T = [h1t_pool.tile([128, OWN], F32R, name=f"h1T{i}")
                   for i in range(4)]

            def layer_norm(dst, src_ps_or_sb, res_sb, g_bc, be_bc, pool):
                """dst = g * norm(src + res) + be (src may be PSUM)."""
                pre = pool.tile([128, D], F32, name="ln_pre")
                nc.vector.tensor_add(out=pre[:], in0=src_ps_or_sb, in1=res_sb)
                stats = pool.tile([128, 6], F32, name="ln_stats")
                nc.vector.bn_stats(out=stats[:], in_=pre[:])
                mv = pool.tile([128, 2], F32, name="ln_mv")
                nc.vector.bn_aggr(out=mv[:], in_=stats[:])
                rstd = pool.tile([128, 1], F32, name="ln_rstd")
                nc.scalar.activation(out=rstd[:], in_=mv[:, 1:2],
                                     func=AF.Sqrt, bias=eps_t[:])
                nc.vector.reciprocal(out=rstd[:], in_=rstd[:])
                nc.vector.tensor_scalar(
                    out=pre[:], in0=pre[:], scalar1=mv[:, 0:1],
                    scalar2=rstd[:], op0=mybir.AluOpType.subtract,
                    op1=mybir.AluOpType.mult)
                nc.vector.tensor_mul(out=pre[:], in0=pre[:], in1=g_bc[:])
                nc.vector.tensor_add(out=dst, in0=pre[:], in1=be_bc[:])

            with tc.tile_pool(name="lnp", bufs=4) as lnp, \
                 tc.tile_pool(name="ps_tr4", bufs=2, space="PSUM") as ps_tr, \
                 tc.tile_pool(name="xrp", bufs=3) as xrp:
                for qt in range(OWN // 128):
                    ored = lnp.tile([128, D], F32, name="ored")
                    nc.sync.dma_start(
                        out=ored, in_=rs_out[qt * 128:(qt + 1) * 128, :])
                    xr = xrp.tile([128, D], F32, name="xr")
                    nc.sync.dma_start(
                        out=xr, in_=xres_d[qt * 128:(qt + 1) * 128, :])
                    layer_norm(h1[qt][:], ored[:], xr[:], g1_bc, be1_bc, lnp)
                    for dc in range(4):
                        trp = ps_tr.tile([128, 128], F32, name="trp")
                        nc.tensor.transpose(
                            trp[:], h1[qt][:, dc * 128:(dc + 1) * 128],
                            ident[:])
                        nc.scalar.copy(
                            out=h1T[dc][:, qt * 128:(qt + 1) * 128],
                            in_=trp[:])

            # ---------------- phase 5: FFN + LN2 + out ----------------
            with tc.tile_pool(name="fw", bufs=1) as fw, \
                 tc.tile_pool(name="ps_f1", bufs=3, space="PSUM") as ps_f1, \
                 tc.tile_pool(name="ps_f2", bufs=1, space="PSUM") as ps_f2, \
                 tc.tile_pool(name="fap", bufs=3) as fap, \
                 tc.tile_pool(name="outp", bufs=3) as outp:
                w1_sb = [fw.tile([128, F], F32R, name=f"w1_{i}") for i in range(4)]
                for dc in range(4):
                    nc.sync.dma_start(
                        out=w1_sb[dc],
                        in_=w1_d[dc * 128:(dc + 1) * 128, :].bitcast(F32R))
                w2_sb = [fw.tile([128, D], F32R, name=f"w2_{i}")
                         for i in range(16)]
                for fc in range(16):
                    nc.sync.dma_start(
                        out=w2_sb[fc],
                        in_=w2_d[fc * 128:(fc + 1) * 128, :].bitcast(F32R))

                for qb2 in range(2):
                    ff2_ps = [ps_f2.tile([128, D], F32, name=f"ff2_{i}")
                              for i in range(4)]
                    for fc in range(16):
                        fp_ps = ps_f1.tile([128, 512], F32, name="fp_ps")
                        for dc in range(4):
                            nc.tensor.matmul(
                                fp_ps[:],
                                w1_sb[dc][:, fc * 128:(fc + 1) * 128],
                                h1T[dc][:, qb2 * 512:(qb2 + 1) * 512],
                                start=(dc == 0), stop=False)
                        nc.tensor.matmul(
                            fp_ps[:], b1_sb[0:1, fc * 128:(fc + 1) * 128],
                            onesr[:], start=False, stop=True)
                        fa = fap.tile([128, 512], F32R, name="fa")
                        nc.scalar.activation(out=fa[:], in_=fp_ps[:],
                                             func=AF.Relu)
                        for qt2 in range(4):
                            nc.tensor.matmul(
                                ff2_ps[qt2][:],
                                fa[:, qt2 * 128:(qt2 + 1) * 128],
                                w2_sb[fc][:], start=(fc == 0), stop=False)
                    for qt2 in range(4):
                        nc.tensor.matmul(ff2_ps[qt2][:], ones128r[:],
                                         b2_sb[:], start=False, stop=True)
                        qt = qb2 * 4 + qt2
                        out_sb = outp.tile([128, D], F32, name="out_sb")
                        layer_norm(out_sb[:], ff2_ps[qt2][:], h1[qt][:],
                                   g2_bc, be2_bc, outp)
                        nc.sync.dma_start(
                            out=out_d[qt * 128:(qt + 1) * 128, :],
                            in_=out_sb[:])

    nc.compile()
    return nc


def _get_nc():
    if "nc" not in _CACHE:
        _CACHE["nc"] = _build()
    return _CACHE["nc"]


def kernel(x, Wq, bq, Wk, bk, Wv, bv, Wo, bo, W1, b1, W2, b2, g1, be1, g2,
           be2, mask=None, **_unused):
    nc = _get_nc()
    in_maps = _make_in_maps(x, Wq, bq, Wk, bk, Wv, bv, Wo, bo, W1, b1, W2, b2,
                            g1, be1, g2, be2)

    res = bass_utils.run_bass_kernel_spmd(
        nc, in_maps, core_ids=list(range(N_CORES)))

    y = np.empty((N, K, D), np.float32)
    for c in range(N_CORES):
        n, s = divmod(c, 2)
        y[n, OWN * s:OWN * s + OWN] = res.results[c]["out"]
    return y


def _make_in_maps(x, Wq, bq, Wk, bk, Wv, bv, Wo, bo, W1, b1, W2, b2, g1, be1,
                  g2, be2):
    x = np.ascontiguousarray(np.asarray(x, dtype=np.float32))
    Wq, Wk, Wv = (np.asarray(w, np.float32) for w in (Wq, Wk, Wv))
    in_maps = []
    for c in range(N_CORES):
        n, s = divmod(c, 2)
        hsel = slice(HH * s, HH * s + HH)
        in_maps.append({
            "xn": x[n],
            "xres": x[n, OWN * s:OWN * s + OWN],
            "wq": np.ascontiguousarray(Wq[hsel].transpose(1, 0, 2).reshape(D, E)),
            "wk": np.ascontiguousarray(Wk[hsel].transpose(1, 0, 2).reshape(D, E)),
            "wv": np.ascontiguousarray(Wv[hsel].transpose(1, 0, 2).reshape(D, E)),
            "bq2": np.ascontiguousarray(np.asarray(bq, np.float32)[hsel]).reshape(1, E),
            "bk2": np.ascontiguousarray(np.asarray(bk, np.float32)[hsel]).reshape(1, E),
            "bv2": np.ascontiguousarray(np.asarray(bv, np.float32)[hsel]).reshape(1, E),
            "wo": np.ascontiguousarray(np.asarray(Wo, np.float32)[E * s:E * s + E]),
            "bo2": (np.asarray(bo, np.float32) * 0.5).reshape(1, D),
            "w1": np.asarray(W1, np.float32),
            "b12": np.asarray(b1, np.float32).reshape(1, F),
            "w2": np.asarray(W2, np.float32),
            "b22": np.asarray(b2, np.float32).reshape(1, D),
            "g1": np.asarray(g1, np.float32).reshape(1, D),
            "be1": np.asarray(be1, np.float32).reshape(1, D),
            "g2": np.asarray(g2, np.float32).reshape(1, D),
            "be2": np.asarray(be2, np.float32).reshape(1, D),
        })
    return in_maps


def kernel_timed(x, Wq, bq, Wk, bk, Wv, bv, Wo, bo, W1, b1, W2, b2, g1, be1,
                 g2, be2, mask=None, **_unused):
    """Run with NTFF tracing; returns BassKernelResults (exec_time_ns etc)."""
    nc = _get_nc()
    in_maps = _make_in_maps(x, Wq, bq, Wk, bk, Wv, bv, Wo, bo, W1, b1, W2, b2,
                            g1, be1, g2, be2)
    return bass_utils.run_bass_kernel_spmd(
        nc, in_maps, core_ids=list(range(N_CORES)), trace=True,
        trace_cores=list(range(N_CORES)))


# revision 10
# speedup vs baseline: 1.2072x; 1.2072x over previous
"""Decoder block (8-head causal attention + FFN + 2x layernorm) on 8 trn2 cores.

Problem: x (4, 2048, 512) fp32; per-head Wq/Wk/Wv (8, 512, 64); Wo (512, 512);
FFN 512->2048->512; causal mask; two post-residual layernorms.

Sharding (uniform SPMD program, 8 cores): core c -> (batch n = c//2,
head-half s = c%2). Each core computes Q/K/V for its 4 heads over the full
2048-token sequence of its batch, causal attention, and its partial Wo
projection (contraction over its 256 attention channels). A pairwise
ReduceScatter sums the two partial Wo outputs and hands each core 1024 rows.
Each core then does residual+LN1, the full FFN (512->2048->512) and
residual+LN2 for its 1024 rows. Host reassembles (4, 2048, 512).

All matmuls run as float32r (TF32-like, 1 cycle/row at N>=256) with fp32 PSUM
accumulation. Causality is exploited: fully-masked key blocks are skipped,
diagonal blocks use one static 128x128 additive triangle mask; softmax runs
without max-subtraction (scores are O(10), exp is safe in fp32) and the
denominator comes for free from an appended ones-column in V (M=65 matmul).
"""

import sys

sys.path.insert(0, "/opt/trn_rl_repo")

import numpy as np

import concourse.bacc as bacc
import concourse.bass as bass
import concourse.mybir as mybir
import concourse.tile as tile
from concourse import bass_utils, masks

F32 = mybir.dt.float32
F32R = mybir.dt.float32r
BF16 = mybir.dt.bfloat16
AF = mybir.ActivationFunctionType

N, K, D, H, F = 4, 2048, 512, 8, 2048
Dh = D // H  # 64
HH = H // 2  # 4 local heads per core
E = HH * Dh  # 256 local attention channels
EPS = 1e-10
N_CORES = 8
OWN = K // 2  # 1024 rows per core after ReduceScatter

_CACHE = {}


def _build():
    nc = bacc.Bacc("TRN2", target_bir_lowering=False, debug=False,
                   num_devices=N_CORES)

    xn_d = nc.dram_tensor("xn", [K, D], F32, kind="ExternalInput")
    xres_d = nc.dram_tensor("xres", [OWN, D], F32, kind="ExternalInput")
    wq_d = nc.dram_tensor("wq", [D, E], F32, kind="ExternalInput")
    wk_d = nc.dram_tensor("wk", [D, E], F32, kind="ExternalInput")
    wv_d = nc.dram_tensor("wv", [D, E], F32, kind="ExternalInput")
    bq_d = nc.dram_tensor("bq2", [1, E], F32, kind="ExternalInput")
    bk_d = nc.dram_tensor("bk2", [1, E], F32, kind="ExternalInput")
    bv_d = nc.dram_tensor("bv2", [1, E], F32, kind="ExternalInput")
    wo_d = nc.dram_tensor("wo", [E, D], F32, kind="ExternalInput")
    bo_d = nc.dram_tensor("bo2", [1, D], F32, kind="ExternalInput")
    w1_d = nc.dram_tensor("w1", [D, F], F32, kind="ExternalInput")
    b1_d = nc.dram_tensor("b12", [1, F], F32, kind="ExternalInput")
    w2_d = nc.dram_tensor("w2", [F, D], F32, kind="ExternalInput")
    b2_d = nc.dram_tensor("b22", [1, D], F32, kind="ExternalInput")
    g1_d = nc.dram_tensor("g1", [1, D], F32, kind="ExternalInput")
    be1_d = nc.dram_tensor("be1", [1, D], F32, kind="ExternalInput")
    g2_d = nc.dram_tensor("g2", [1, D], F32, kind="ExternalInput")
    be2_d = nc.dram_tensor("be2", [1, D], F32, kind="ExternalInput")
    out_d = nc.dram_tensor("out", [OWN, D], F32, kind="ExternalOutput")

    def bcast(dram, n):
        # [1, n] DRAM row broadcast to [128, n]
        return bass.AP(tensor=dram, offset=0, ap=[[0, 128], [1, n]])

    with tile.TileContext(nc) as tc:
        import contextlib
        stack = contextlib.ExitStack()
        with stack:
            singles = stack.enter_context(tc.tile_pool(name="singles", bufs=1))
            dram = stack.enter_context(
                tc.tile_pool(name="dram", bufs=1, space="DRAM"))

            # ---- static tiles ----
            ident = singles.tile([128, 128], F32)
            masks.make_identity(nc, ident[:])
            tri01 = singles.tile([128, 128], BF16)
            nc.gpsimd.memset(tri01, 1.0)
            # keep 1.0 where q - k >= 0 (k<=q), else 0 (partition = key, free = query)
            nc.gpsimd.affine_select(
                out=tri01, in_=tri01, compare_op=mybir.AluOpType.is_ge,
                fill=0.0, base=0, pattern=[[1, 128]], channel_multiplier=-1)
            ones_f32 = singles.tile([128, 64], F32)
            nc.vector.memset(ones_f32, 1.0)
            ones64r = singles.tile([1, 64], F32R)
            nc.vector.tensor_copy(out=ones64r[:], in_=ones_f32[0:1, :])
            ones4 = singles.tile([128, HH, 1], BF16)
            nc.vector.memset(ones4, 1.0)
            eps_t = singles.tile([128, 1], F32)
            nc.vector.memset(eps_t, EPS)

            # gains/biases broadcast to 128 partitions
            g1_bc = singles.tile([128, D], F32)
            nc.gpsimd.dma_start(out=g1_bc, in_=bcast(g1_d, D))
            be1_bc = singles.tile([128, D], F32)
            nc.gpsimd.dma_start(out=be1_bc, in_=bcast(be1_d, D))
            g2_bc = singles.tile([128, D], F32)
            nc.gpsimd.dma_start(out=g2_bc, in_=bcast(g2_d, D))
            be2_bc = singles.tile([128, D], F32)
            nc.gpsimd.dma_start(out=be2_bc, in_=bcast(be2_d, D))

            # biases: per-partition columns (for ACT bias) and broadcasts
            bq_col = singles.tile([128, 2], F32)
            nc.gpsimd.dma_start(out=bq_col, in_=bass.AP(
                tensor=bq_d, offset=0, ap=[[1, 128], [128, 2]]))
            bk_col = singles.tile([128, 2], F32)
            nc.gpsimd.dma_start(out=bk_col, in_=bass.AP(
                tensor=bk_d, offset=0, ap=[[1, 128], [128, 2]]))
            b1_col = singles.tile([128, 16], F32)
            nc.gpsimd.dma_start(out=b1_col, in_=bass.AP(
                tensor=b1_d, offset=0, ap=[[1, 128], [128, 16]]))
            bv_bc = singles.tile([128, HH, Dh], F32)
            nc.gpsimd.dma_start(out=bv_bc, in_=bass.AP(
                tensor=bv_d, offset=0, ap=[[0, 128], [64, HH], [1, Dh]]))
            bo_bc = singles.tile([128, D], F32)
            nc.gpsimd.dma_start(out=bo_bc, in_=bcast(bo_d, D))
            b2_bc = singles.tile([128, D], F32)
            nc.gpsimd.dma_start(out=b2_bc, in_=bcast(b2_d, D))

            # persistent activation tensors
            kt_pool = stack.enter_context(tc.tile_pool(name="kt", bufs=1))
            qt_pool = stack.enter_context(tc.tile_pool(name="qt", bufs=1))
            va_pool = stack.enter_context(tc.tile_pool(name="va", bufs=1))
            ac_pool = stack.enter_context(tc.tile_pool(name="ac", bufs=1))
            kT = [kt_pool.tile([128, K], BF16, name=f"kT{i}") for i in range(2)]
            qT = [qt_pool.tile([128, K], BF16, name=f"qT{i}") for i in range(2)]
            va = [va_pool.tile([128, HH, Dh + 1], BF16, name=f"va{i}")
                  for i in range(K // 128)]
            ac = [ac_pool.tile([128, K], F32R, name=f"ac{i}") for i in range(2)]

            # ---------------- phase 1: xT + projections ----------------
            with tc.tile_pool(name="pw", bufs=1) as pw, \
                 tc.tile_pool(name="xp", bufs=4) as xp, \
                 tc.tile_pool(name="xt", bufs=1) as xtp, \
                 tc.tile_pool(name="ps_tr1", bufs=2, space="PSUM") as ps_tr, \
                 tc.tile_pool(name="ps_proj", bufs=3, space="PSUM") as ps_proj:
                wq_sb = [pw.tile([128, E], F32R, name=f"wq{i}") for i in range(4)]
                wk_sb = [pw.tile([128, E], F32R, name=f"wk{i}") for i in range(4)]
                wv_sb = [pw.tile([128, E], F32R, name=f"wv{i}") for i in range(4)]
                for dc in range(4):
                    nc.sync.dma_start(
                        out=wq_sb[dc],
                        in_=wq_d[dc * 128:(dc + 1) * 128, :].bitcast(F32R))
                    nc.sync.dma_start(
                        out=wk_sb[dc],
                        in_=wk_d[dc * 128:(dc + 1) * 128, :].bitcast(F32R))
                    nc.sync.dma_start(
                        out=wv_sb[dc],
                        in_=wv_d[dc * 128:(dc + 1) * 128, :].bitcast(F32R))

                xT = [xtp.tile([128, K], F32R, name=f"xT{i}") for i in range(4)]
                for kt_i in range(K // 128):
                    xrow = xp.tile([128, D], F32, name="xrow")
                    nc.sync.dma_start(
                        out=xrow, in_=xn_d[kt_i * 128:(kt_i + 1) * 128, :])
                    for dc in range(4):
                        trp = ps_tr.tile([128, 128], F32, name="trp")
                        nc.tensor.transpose(
                            trp[:], xrow[:, dc * 128:(dc + 1) * 128], ident[:])
                        nc.scalar.copy(
                            out=xT[dc][:, kt_i * 128:(kt_i + 1) * 128],
                            in_=trp[:])

                # kT / qT: per head-pair hp, 512-wide key/query block kb
                for w_sb, b_col, dstT in ((wk_sb, bk_col, kT), (wq_sb, bq_col, qT)):
                    for hp in range(2):
                        for kb in range(4):
                            pp = ps_proj.tile([128, 512], F32, name="pp")
                            for dc in range(4):
                                nc.tensor.matmul(
                                    pp[:],
                                    w_sb[dc][:, hp * 128:(hp + 1) * 128],
                                    xT[dc][:, kb * 512:(kb + 1) * 512],
                                    start=(dc == 0), stop=(dc == 3))
                            nc.scalar.activation(
                                out=dstT[hp][:, kb * 512:(kb + 1) * 512],
                                in_=pp[:], func=AF.Identity,
                                bias=b_col[:, hp:hp + 1])

                # v rows (all 4 heads at once), augmented with ones column
                for kt_i in range(K // 128):
                    vp = ps_proj.tile([128, E], F32, name="vp")
                    for dc in range(4):
                        nc.tensor.matmul(
                            vp[:],
                            xT[dc][:, kt_i * 128:(kt_i + 1) * 128],
                            wv_sb[dc][:], start=(dc == 0), stop=(dc == 3))
                    nc.vector.tensor_add(
                        out=va[kt_i][:, :, 0:Dh],
                        in0=vp[:].rearrange("p (h e) -> p h e", h=HH),
                        in1=bv_bc[:])
                    nc.vector.tensor_copy(out=va[kt_i][:, :, Dh:Dh + 1],
                                          in_=ones4[:])

            # ---------------- phase 2: causal attention ----------------
            with tc.tile_pool(name="ps_s", bufs=3, space="PSUM") as ps_s, \
                 tc.tile_pool(name="ps_att", bufs=2, space="PSUM") as ps_att, \
                 tc.tile_pool(name="ps_bc", bufs=2, space="PSUM") as ps_bc, \
                 tc.tile_pool(name="expp", bufs=4) as expp, \
                 tc.tile_pool(name="bcp", bufs=3) as bcp:
                for h in range(HH):
                    hp, h2 = divmod(h, 2)
                    erow = slice(h2 * 64, h2 * 64 + 64)
                    for qb in range(4):
                        qs = qb * 512
                        att_ps = ps_att.tile([65, 512], F32, name="att_ps")
                        n_mm = 0
                        for kb in range(4 * qb):  # full key blocks
                            s_ps = ps_s.tile([128, 512], F32, name="s_ps")
                            nc.tensor.matmul(
                                s_ps[:],
                                kT[hp][erow, kb * 128:(kb + 1) * 128],
                                qT[hp][erow, qs:qs + 512],
                                start=True, stop=True)
                            expT = expp.tile([128, 512], BF16, name="expT")
                            nc.scalar.activation(out=expT[:], in_=s_ps[:],
                                                 func=AF.Exp, scale=0.125)
                            nc.tensor.matmul(
                                att_ps[:], va[kb][:, h, :], expT[:],
                                start=(n_mm == 0), stop=False)
                            n_mm += 1
                        for m in range(4):  # diagonal key blocks
                            kb = 4 * qb + m
                            lo = m * 128
                            s_ps = ps_s.tile([128, 512], F32, name="s_ps")
                            nc.tensor.matmul(
                                s_ps[:, lo:512],
                                kT[hp][erow, kb * 128:(kb + 1) * 128],
                                qT[hp][erow, qs + lo:qs + 512],
                                start=True, stop=True)
                            expT = expp.tile([128, 512], BF16, name="expT")
                            nc.scalar.activation(out=expT[:, lo:512],
                                                 in_=s_ps[:, lo:512],
                                                 func=AF.Exp, scale=0.125)
                            # zero the still-masked triangle (k > q)
                            nc.gpsimd.tensor_mul(
                                out=expT[:, lo:lo + 128],
                                in0=expT[:, lo:lo + 128], in1=tri01[:])
                            nc.tensor.matmul(
                                att_ps[:, lo:512], va[kb][:, h, :],
                                expT[:, lo:512],
                                start=(n_mm == 0), stop=(m == 3))
                            n_mm += 1
                        # softmax normalize: broadcast denom row, then divide
                        den_sb = bcp.tile([1, 512], F32R, name="den_sb")
                        nc.scalar.copy(out=den_sb[:], in_=att_ps[64:65, :])
                        bc_ps = ps_bc.tile([64, 512], F32, name="bc_ps")
                        nc.tensor.matmul(bc_ps[:], ones64r[:], den_sb[:],
                                         start=True, stop=True)
                        bc_rec = bcp.tile([64, 512], F32, name="bc_rec")
                        nc.vector.reciprocal(out=bc_rec[:], in_=bc_ps[:])
                        with nc.allow_low_precision(reason="f32r attn out"):
                            nc.vector.tensor_mul(
                                out=ac[hp][erow, qs:qs + 512],
                                in0=att_ps[0:64, :], in1=bc_rec[:])

            # ---------------- phase 3: Wo partial + ReduceScatter ----------
            rs_in = dram.tile([K, D], F32, name="rs_in")
            rs_out = dram.tile([OWN, D], F32, name="rs_out")
            with tc.tile_pool(name="wop", bufs=1) as wop, \
                 tc.tile_pool(name="ps_o", bufs=3, space="PSUM") as ps_o, \
                 tc.tile_pool(name="op", bufs=3) as op:
                wo_sb = [wop.tile([128, D], F32R, name=f"wo{i}") for i in range(2)]
                for hp in range(2):
                    nc.sync.dma_start(
                        out=wo_sb[hp],
                        in_=wo_d[hp * 128:(hp + 1) * 128, :].bitcast(F32R))
                for qt in range(K // 128):
                    o_ps = ps_o.tile([128, D], F32, name="o_ps")
                    for hp in range(2):
                        nc.tensor.matmul(
                            o_ps[:], ac[hp][:, qt * 128:(qt + 1) * 128],
                            wo_sb[hp][:], start=(hp == 0), stop=(hp == 1))
                    o_sb = op.tile([128, D], F32, name="o_sb")
                    nc.vector.tensor_add(out=o_sb[:], in0=o_ps[:], in1=bo_bc[:])
                    nc.sync.dma_start(
                        out=rs_in[qt * 128:(qt + 1) * 128, :], in_=o_sb[:])
            nc.gpsimd.collective_compute(
                "ReduceScatter", mybir.AluOpType.add,
                replica_groups=[[0, 1], [2, 3], [4, 5], [6, 7]],
                ins=[rs_in[:]], outs=[rs_out[:]])

            # ---------------- phase 4: residual + LN1 + h1T ----------------
            h1_pool = stack.enter_context(tc.tile_pool(name="h1", bufs=1))
            h1t_pool = stack.enter_context(tc.tile_pool(name="h1t", bufs=1))
            h1 = [h1_pool.tile([128, D], F32, name=f"h1_{i}")
                  for i in range(OWN // 128)]
            h1T = [h1t_pool.tile([128, OWN], F32R, name=f"h1T{i}")
                   for i in range(4)]

            def layer_norm(dst, src_ps_or_sb, res_sb, g_bc, be_bc, pool,
                           extra_bc=None):
                """dst = g * norm(src + res [+ extra]) + be (src may be PSUM)."""
                pre = pool.tile([128, D], F32, name="ln_pre")
                nc.vector.tensor_add(out=pre[:], in0=src_ps_or_sb, in1=res_sb)
                if extra_bc is not None:
                    nc.vector.tensor_add(out=pre[:], in0=pre[:], in1=extra_bc[:])
                stats = pool.tile([128, 6], F32, name="ln_stats")
                nc.vector.bn_stats(out=stats[:], in_=pre[:])
                mv = pool.tile([128, 2], F32, name="ln_mv")
                nc.vector.bn_aggr(out=mv[:], in_=stats[:])
                rstd = pool.tile([128, 1], F32, name="ln_rstd")
                nc.scalar.activation(out=rstd[:], in_=mv[:, 1:2],
                                     func=AF.Sqrt, bias=eps_t[:])
                nc.vector.reciprocal(out=rstd[:], in_=rstd[:])
                nc.vector.tensor_scalar(
                    out=pre[:], in0=pre[:], scalar1=mv[:, 0:1],
                    scalar2=rstd[:], op0=mybir.AluOpType.subtract,
                    op1=mybir.AluOpType.mult)
                nc.vector.tensor_mul(out=pre[:], in0=pre[:], in1=g_bc[:])
                nc.vector.tensor_add(out=dst, in0=pre[:], in1=be_bc[:])

            with tc.tile_pool(name="lnp", bufs=4) as lnp, \
                 tc.tile_pool(name="ps_tr4", bufs=2, space="PSUM") as ps_tr, \
                 tc.tile_pool(name="xrp", bufs=3) as xrp:
                for qt in range(OWN // 128):
                    ored = lnp.tile([128, D], F32, name="ored")
                    nc.sync.dma_start(
                        out=ored, in_=rs_out[qt * 128:(qt + 1) * 128, :])
                    xr = xrp.tile([128, D], F32, name="xr")
                    nc.sync.dma_start(
                        out=xr, in_=xres_d[qt * 128:(qt + 1) * 128, :])
                    layer_norm(h1[qt][:], ored[:], xr[:], g1_bc, be1_bc, lnp)
                    for dc in range(4):
                        trp = ps_tr.tile([128, 128], F32, name="trp")
                        nc.tensor.transpose(
                            trp[:], h1[qt][:, dc * 128:(dc + 1) * 128],
                            ident[:])
                        nc.scalar.copy(
                            out=h1T[dc][:, qt * 128:(qt + 1) * 128],
                            in_=trp[:])

            # ---------------- phase 5: FFN + LN2 + out ----------------
            with tc.tile_pool(name="fw", bufs=1) as fw, \
                 tc.tile_pool(name="ps_f1", bufs=3, space="PSUM") as ps_f1, \
                 tc.tile_pool(name="ps_f2", bufs=1, space="PSUM") as ps_f2, \
                 tc.tile_pool(name="fap", bufs=3) as fap, \
                 tc.tile_pool(name="outp", bufs=3) as outp:
                w1_sb = [fw.tile([128, F], F32R, name=f"w1_{i}") for i in range(4)]
                for dc in range(4):
                    nc.sync.dma_start(
                        out=w1_sb[dc],
                        in_=w1_d[dc * 128:(dc + 1) * 128, :].bitcast(F32R))
                w2_sb = [fw.tile([128, D], F32R, name=f"w2_{i}")
                         for i in range(16)]
                for fc in range(16):
                    nc.sync.dma_start(
                        out=w2_sb[fc],
                        in_=w2_d[fc * 128:(fc + 1) * 128, :].bitcast(F32R))

                for qb2 in range(2):
                    ff2_ps = [ps_f2.tile([128, D], F32, name=f"ff2_{i}")
                              for i in range(4)]
                    for fc in range(16):
                        fp_ps = ps_f1.tile([128, 512], F32, name="fp_ps")
                        for dc in range(4):
                            nc.tensor.matmul(
                                fp_ps[:],
                                w1_sb[dc][:, fc * 128:(fc + 1) * 128],
                                h1T[dc][:, qb2 * 512:(qb2 + 1) * 512],
                                start=(dc == 0), stop=(dc == 3))
                        fa = fap.tile([128, 512], F32R, name="fa")
                        nc.scalar.activation(out=fa[:], in_=fp_ps[:],
                                             func=AF.Relu,
                                             bias=b1_col[:, fc:fc + 1])
                        for qt2 in range(4):
                            nc.tensor.matmul(
                                ff2_ps[qt2][:],
                                fa[:, qt2 * 128:(qt2 + 1) * 128],
                                w2_sb[fc][:], start=(fc == 0), stop=(fc == 15))
                    for qt2 in range(4):
                        qt = qb2 * 4 + qt2
                        out_sb = outp.tile([128, D], F32, name="out_sb")
                        layer_norm(out_sb[:], ff2_ps[qt2][:], h1[qt][:],
                                   g2_bc, be2_bc, outp, extra_bc=b2_bc)
                        nc.sync.dma_start(
                            out=out_d[qt * 128:(qt + 1) * 128, :],
                            in_=out_sb[:])

    nc.compile()
    return nc


def _get_nc():
    if "nc" not in _CACHE:
        _CACHE["nc"] = _build()
    return _CACHE["nc"]


def kernel(x, Wq, bq, Wk, bk, Wv, bv, Wo, bo, W1, b1, W2, b2, g1, be1, g2,
           be2, mask=None, **_unused):
    nc = _get_nc()
    in_maps = _make_in_maps(x, Wq, bq, Wk, bk, Wv, bv, Wo, bo, W1, b1, W2, b2,
                            g1, be1, g2, be2)

    res = bass_utils.run_bass_kernel_spmd(
        nc, in_maps, core_ids=list(range(N_CORES)))

    y = np.empty((N, K, D), np.float32)
    for c in range(N_CORES):
        n, s = divmod(c, 2)
        y[n, OWN * s:OWN * s + OWN] = res.results[c]["out"]
    return y


def _make_in_maps(x, Wq, bq, Wk, bk, Wv, bv, Wo, bo, W1, b1, W2, b2, g1, be1,
                  g2, be2):
    x = np.ascontiguousarray(np.asarray(x, dtype=np.float32))
    Wq, Wk, Wv = (np.asarray(w, np.float32) for w in (Wq, Wk, Wv))
    in_maps = []
    for c in range(N_CORES):
        n, s = divmod(c, 2)
        hsel = slice(HH * s, HH * s + HH)
        in_maps.append({
            "xn": x[n],
            "xres": x[n, OWN * s:OWN * s + OWN],
            "wq": np.ascontiguousarray(Wq[hsel].transpose(1, 0, 2).reshape(D, E)),
            "wk": np.ascontiguousarray(Wk[hsel].transpose(1, 0, 2).reshape(D, E)),
            "wv": np.ascontiguousarray(Wv[hsel].transpose(1, 0, 2).reshape(D, E)),
            "bq2": np.ascontiguousarray(np.asarray(bq, np.float32)[hsel]).reshape(1, E),
            "bk2": np.ascontiguousarray(np.asarray(bk, np.float32)[hsel]).reshape(1, E),
            "bv2": np.ascontiguousarray(np.asarray(bv, np.float32)[hsel]).reshape(1, E),
            "wo": np.ascontiguousarray(np.asarray(Wo, np.float32)[E * s:E * s + E]),
            "bo2": (np.asarray(bo, np.float32) * 0.5).reshape(1, D),
            "w1": np.asarray(W1, np.float32),
            "b12": np.asarray(b1, np.float32).reshape(1, F),
            "w2": np.asarray(W2, np.float32),
            "b22": np.asarray(b2, np.float32).reshape(1, D),
            "g1": np.asarray(g1, np.float32).reshape(1, D),
            "be1": np.asarray(be1, np.float32).reshape(1, D),
            "g2": np.asarray(g2, np.float32).reshape(1, D),
            "be2": np.asarray(be2, np.float32).reshape(1, D),
        })
    return in_maps


def kernel_timed(x, Wq, bq, Wk, bk, Wv, bv, Wo, bo, W1, b1, W2, b2, g1, be1,
                 g2, be2, mask=None, **_unused):
    """Run with NTFF tracing; returns BassKernelResults (exec_time_ns etc)."""
    nc = _get_nc()
    in_maps = _make_in_maps(x, Wq, bq, Wk, bk, Wv, bv, Wo, bo, W1, b1, W2, b2,
                            g1, be1, g2, be2)
    return bass_utils.run_bass_kernel_spmd(
        nc, in_maps, core_ids=list(range(N_CORES)), trace=True,
        trace_cores=list(range(N_CORES)))


# revision 13
# speedup vs baseline: 1.2339x; 1.0221x over previous
"""Decoder block (8-head causal attention + FFN + 2x layernorm) on 8 trn2 cores.

Problem: x (4, 2048, 512) fp32; per-head Wq/Wk/Wv (8, 512, 64); Wo (512, 512);
FFN 512->2048->512; causal mask; two post-residual layernorms.

Sharding (uniform SPMD program, 8 cores): core c -> (batch n = c//2,
head-half s = c%2). Each core computes Q/K/V for its 4 heads over the full
2048-token sequence of its batch, causal attention, and its partial Wo
projection (contraction over its 256 attention channels). A pairwise
ReduceScatter sums the two partial Wo outputs and hands each core 1024 rows.
Each core then does residual+LN1, the full FFN (512->2048->512) and
residual+LN2 for its 1024 rows. Host reassembles (4, 2048, 512).

All matmuls run as float32r (TF32-like, 1 cycle/row at N>=256) with fp32 PSUM
accumulation. Causality is exploited: fully-masked key blocks are skipped,
diagonal blocks use one static 128x128 additive triangle mask; softmax runs
without max-subtraction (scores are O(10), exp is safe in fp32) and the
denominator comes for free from an appended ones-column in V (M=65 matmul).
"""

import sys

sys.path.insert(0, "/opt/trn_rl_repo")

import numpy as np

import concourse.bacc as bacc
import concourse.bass as bass
import concourse.mybir as mybir
import concourse.tile as tile
from concourse import bass_utils, masks

F32 = mybir.dt.float32
F32R = mybir.dt.float32r
BF16 = mybir.dt.bfloat16
AF = mybir.ActivationFunctionType

N, K, D, H, F = 4, 2048, 512, 8, 2048
Dh = D // H  # 64
HH = H // 2  # 4 local heads per core
E = HH * Dh  # 256 local attention channels
EPS = 1e-10
N_CORES = 8
OWN = K // 2  # 1024 rows per core after ReduceScatter

_CACHE = {}


def _build():
    nc = bacc.Bacc("TRN2", target_bir_lowering=False, debug=False,
                   num_devices=N_CORES)

    xn_d = nc.dram_tensor("xn", [K, D], F32, kind="ExternalInput")
    xres_d = nc.dram_tensor("xres", [OWN, D], F32, kind="ExternalInput")
    wq_d = nc.dram_tensor("wq", [D, E], F32, kind="ExternalInput")
    wk_d = nc.dram_tensor("wk", [D, E], F32, kind="ExternalInput")
    wv_d = nc.dram_tensor("wv", [D, E], F32, kind="ExternalInput")
    bq_d = nc.dram_tensor("bq2", [1, E], F32, kind="ExternalInput")
    bk_d = nc.dram_tensor("bk2", [1, E], F32, kind="ExternalInput")
    bv_d = nc.dram_tensor("bv2", [1, E], F32, kind="ExternalInput")
    wo_d = nc.dram_tensor("wo", [E, D], F32, kind="ExternalInput")
    bo_d = nc.dram_tensor("bo2", [1, D], F32, kind="ExternalInput")
    w1_d = nc.dram_tensor("w1", [D, F], F32, kind="ExternalInput")
    b1_d = nc.dram_tensor("b12", [1, F], F32, kind="ExternalInput")
    w2_d = nc.dram_tensor("w2", [F, D], F32, kind="ExternalInput")
    b2_d = nc.dram_tensor("b22", [1, D], F32, kind="ExternalInput")
    g1_d = nc.dram_tensor("g1", [1, D], F32, kind="ExternalInput")
    be1_d = nc.dram_tensor("be1", [1, D], F32, kind="ExternalInput")
    g2_d = nc.dram_tensor("g2", [1, D], F32, kind="ExternalInput")
    be2_d = nc.dram_tensor("be2", [1, D], F32, kind="ExternalInput")
    out_d = nc.dram_tensor("out", [OWN, D], F32, kind="ExternalOutput")

    def bcast(dram, n):
        # [1, n] DRAM row broadcast to [128, n]
        return bass.AP(tensor=dram, offset=0, ap=[[0, 128], [1, n]])

    with tile.TileContext(nc) as tc:
        import contextlib
        stack = contextlib.ExitStack()
        with stack:
            singles = stack.enter_context(tc.tile_pool(name="singles", bufs=1))
            dram = stack.enter_context(
                tc.tile_pool(name="dram", bufs=1, space="DRAM"))

            # ---- static tiles ----
            ident = singles.tile([128, 128], F32)
            masks.make_identity(nc, ident[:])
            tri01 = singles.tile([128, 128], BF16)
            nc.gpsimd.memset(tri01, 1.0)
            # keep 1.0 where q - k >= 0 (k<=q), else 0 (partition = key, free = query)
            nc.gpsimd.affine_select(
                out=tri01, in_=tri01, compare_op=mybir.AluOpType.is_ge,
                fill=0.0, base=0, pattern=[[1, 128]], channel_multiplier=-1)
            ones_f32 = singles.tile([128, 64], F32)
            nc.vector.memset(ones_f32, 1.0)
            ones64r = singles.tile([1, 64], F32R)
            nc.vector.tensor_copy(out=ones64r[:], in_=ones_f32[0:1, :])
            ones4 = singles.tile([128, HH, 1], BF16)
            nc.vector.memset(ones4, 1.0)
            eps_t = singles.tile([128, 1], F32)
            nc.vector.memset(eps_t, EPS)

            # gains/biases broadcast to 128 partitions
            g1_bc = singles.tile([128, D], F32)
            nc.gpsimd.dma_start(out=g1_bc, in_=bcast(g1_d, D))
            be1_bc = singles.tile([128, D], F32)
            nc.gpsimd.dma_start(out=be1_bc, in_=bcast(be1_d, D))
            g2_bc = singles.tile([128, D], F32)
            nc.gpsimd.dma_start(out=g2_bc, in_=bcast(g2_d, D))
            be2_bc = singles.tile([128, D], F32)
            nc.gpsimd.dma_start(out=be2_bc, in_=bcast(be2_d, D))

            # biases: per-partition columns (for ACT bias) and broadcasts
            bq_col = singles.tile([128, 2], F32)
            nc.gpsimd.dma_start(out=bq_col, in_=bass.AP(
                tensor=bq_d, offset=0, ap=[[1, 128], [128, 2]]))
            bk_col = singles.tile([128, 2], F32)
            nc.gpsimd.dma_start(out=bk_col, in_=bass.AP(
                tensor=bk_d, offset=0, ap=[[1, 128], [128, 2]]))
            b1_col = singles.tile([128, 16], F32)
            nc.gpsimd.dma_start(out=b1_col, in_=bass.AP(
                tensor=b1_d, offset=0, ap=[[1, 128], [128, 16]]))
            bv_bc = singles.tile([128, HH, Dh], F32)
            nc.gpsimd.dma_start(out=bv_bc, in_=bass.AP(
                tensor=bv_d, offset=0, ap=[[0, 128], [64, HH], [1, Dh]]))
            bo_bc = singles.tile([128, D], F32)
            nc.gpsimd.dma_start(out=bo_bc, in_=bcast(bo_d, D))
            b2_bc = singles.tile([128, D], F32)
            nc.gpsimd.dma_start(out=b2_bc, in_=bcast(b2_d, D))

            # persistent activation tensors
            kt_pool = stack.enter_context(tc.tile_pool(name="kt", bufs=1))
            qt_pool = stack.enter_context(tc.tile_pool(name="qt", bufs=1))
            va_pool = stack.enter_context(tc.tile_pool(name="va", bufs=1))
            ac_pool = stack.enter_context(tc.tile_pool(name="ac", bufs=1))
            kT = [kt_pool.tile([128, K], BF16, name=f"kT{i}") for i in range(2)]
            qT = [qt_pool.tile([128, K], BF16, name=f"qT{i}") for i in range(2)]
            va = [va_pool.tile([128, HH, Dh + 1], BF16, name=f"va{i}")
                  for i in range(K // 128)]
            ac = [ac_pool.tile([128, K], F32R, name=f"ac{i}") for i in range(2)]

            # ---------------- phase 1: xT + projections ----------------
            with tc.tile_pool(name="pw", bufs=1) as pw, \
                 tc.tile_pool(name="xp", bufs=4) as xp, \
                 tc.tile_pool(name="xt", bufs=1) as xtp, \
                 tc.tile_pool(name="ps_tr1", bufs=2, space="PSUM") as ps_tr, \
                 tc.tile_pool(name="ps_proj", bufs=3, space="PSUM") as ps_proj:
                wq_sb = [pw.tile([128, E], F32R, name=f"wq{i}") for i in range(4)]
                wk_sb = [pw.tile([128, E], F32R, name=f"wk{i}") for i in range(4)]
                wv_sb = [pw.tile([128, E], F32R, name=f"wv{i}") for i in range(4)]
                for dc in range(4):
                    nc.sync.dma_start(
                        out=wq_sb[dc],
                        in_=wq_d[dc * 128:(dc + 1) * 128, :].bitcast(F32R))
                    nc.sync.dma_start(
                        out=wk_sb[dc],
                        in_=wk_d[dc * 128:(dc + 1) * 128, :].bitcast(F32R))
                    nc.sync.dma_start(
                        out=wv_sb[dc],
                        in_=wv_d[dc * 128:(dc + 1) * 128, :].bitcast(F32R))

                xT = [xtp.tile([128, K], F32R, name=f"xT{i}") for i in range(4)]
                for kt_i in range(K // 128):
                    xrow = xp.tile([128, D], F32, name="xrow")
                    nc.sync.dma_start(
                        out=xrow, in_=xn_d[kt_i * 128:(kt_i + 1) * 128, :])
                    for dc in range(4):
                        trp = ps_tr.tile([128, 128], F32, name="trp")
                        nc.tensor.transpose(
                            trp[:], xrow[:, dc * 128:(dc + 1) * 128], ident[:])
                        nc.scalar.copy(
                            out=xT[dc][:, kt_i * 128:(kt_i + 1) * 128],
                            in_=trp[:])

                # kT / qT: per head-pair hp, 512-wide key/query block kb
                for w_sb, b_col, dstT in ((wk_sb, bk_col, kT), (wq_sb, bq_col, qT)):
                    for hp in range(2):
                        for kb in range(4):
                            pp = ps_proj.tile([128, 512], F32, name="pp")
                            for dc in range(4):
                                nc.tensor.matmul(
                                    pp[:],
                                    w_sb[dc][:, hp * 128:(hp + 1) * 128],
                                    xT[dc][:, kb * 512:(kb + 1) * 512],
                                    start=(dc == 0), stop=(dc == 3))
                            nc.scalar.activation(
                                out=dstT[hp][:, kb * 512:(kb + 1) * 512],
                                in_=pp[:], func=AF.Identity,
                                bias=b_col[:, hp:hp + 1])

                # v rows (all 4 heads at once), augmented with ones column
                for kt_i in range(K // 128):
                    vp = ps_proj.tile([128, E], F32, name="vp")
                    for dc in range(4):
                        nc.tensor.matmul(
                            vp[:],
                            xT[dc][:, kt_i * 128:(kt_i + 1) * 128],
                            wv_sb[dc][:], start=(dc == 0), stop=(dc == 3))
                    nc.vector.tensor_add(
                        out=va[kt_i][:, :, 0:Dh],
                        in0=vp[:].rearrange("p (h e) -> p h e", h=HH),
                        in1=bv_bc[:])
                    nc.vector.tensor_copy(out=va[kt_i][:, :, Dh:Dh + 1],
                                          in_=ones4[:])

            # ---------------- phase 2: causal attention ----------------
            with tc.tile_pool(name="ps_s", bufs=2, space="PSUM") as ps_s, \
                 tc.tile_pool(name="ps_att", bufs=4, space="PSUM") as ps_att, \
                 tc.tile_pool(name="expp", bufs=4) as expp, \
                 tc.tile_pool(name="bcp", bufs=2) as bcp, \
                 tc.tile_pool(name="bcs", bufs=4) as bcs:
                for qb in range(4):
                    qs = qb * 512
                    att_list = []
                    for h in range(HH):
                        hp, h2 = divmod(h, 2)
                        erow = slice(h2 * 64, h2 * 64 + 64)
                        att_ps = ps_att.tile([65, 512], F32, name="att_ps")
                        att_list.append((att_ps, hp, erow))
                        n_mm = 0
                        # full key blocks, two at a time sharing one exp
                        for p in range(2 * qb):
                            kb0, kb1 = 2 * p, 2 * p + 1
                            s2 = ps_s.tile([128, 1024], F32, name="s2")
                            for j, kb in enumerate((kb0, kb1)):
                                nc.tensor.matmul(
                                    s2[:, j * 512:(j + 1) * 512],
                                    kT[hp][erow, kb * 128:(kb + 1) * 128],
                                    qT[hp][erow, qs:qs + 512],
                                    start=True, stop=True)
                            expT = expp.tile([128, 1024], BF16, name="expT")
                            nc.scalar.activation(out=expT[:], in_=s2[:],
                                                 func=AF.Exp, scale=0.125)
                            for j, kb in enumerate((kb0, kb1)):
                                nc.tensor.matmul(
                                    att_ps[:], va[kb][:, h, :],
                                    expT[:, j * 512:(j + 1) * 512],
                                    start=(n_mm == 0), stop=False)
                                n_mm += 1
                        for m in range(4):  # diagonal key blocks
                            kb = 4 * qb + m
                            lo = m * 128
                            s2 = ps_s.tile([128, 1024], F32, name="s2")
                            nc.tensor.matmul(
                                s2[:, lo:512],
                                kT[hp][erow, kb * 128:(kb + 1) * 128],
                                qT[hp][erow, qs + lo:qs + 512],
                                start=True, stop=True)
                            expT = expp.tile([128, 1024], BF16, name="expT")
                            nc.scalar.activation(out=expT[:, lo:512],
                                                 in_=s2[:, lo:512],
                                                 func=AF.Exp, scale=0.125)
                            # zero the still-masked triangle (k > q)
                            nc.vector.tensor_mul(
                                out=expT[:, lo:lo + 128],
                                in0=expT[:, lo:lo + 128], in1=tri01[:])
                            nc.tensor.matmul(
                                att_ps[:, lo:512], va[kb][:, h, :],
                                expT[:, lo:512],
                                start=(n_mm == 0), stop=(m == 3))
                            n_mm += 1
                    # softmax normalize for all 4 heads of this q block:
                    # gather denom rows, one reciprocal, DMA-broadcast, mul
                    # engine writes must start at 32-aligned partitions:
                    # park head h's denominator row on partition 32*h
                    den4 = bcp.tile([128, 512], F32, name="den4")
                    for h, (att_ps, hp, erow) in enumerate(att_list):
                        nc.scalar.copy(out=den4[32 * h:32 * h + 1, :],
                                       in_=att_ps[64:65, :])
                    rec4 = bcp.tile([128, 512], F32, name="rec4")
                    nc.vector.reciprocal(out=rec4[0:97, :], in_=den4[0:97, :])
                    rec_dr = dram.tile([HH, 512], F32, name="rec_dr")
                    nc.sync.dma_start(
                        out=rec_dr[:],
                        in_=rec4[:].rearrange("(a b) f -> a b f", b=32)[:, 0, :])
                    for h, (att_ps, hp, erow) in enumerate(att_list):
                        bc_sb = bcs.tile([64, 512], F32, name="bc_sb")
                        nc.sync.dma_start(out=bc_sb[:], in_=bass.AP(
                            tensor=rec_dr[:].tensor, offset=h * 512,
                            ap=[[0, 64], [1, 512]]))
                        nc.vector.tensor_mul(
                            out=ac[hp][erow, qs:qs + 512],
                            in0=att_ps[0:64, :], in1=bc_sb[:])

            # ---------------- phase 3: Wo partial + ReduceScatter ----------
            rs_in = dram.tile([K, D], F32, name="rs_in")
            rs_out = dram.tile([OWN, D], F32, name="rs_out")
            with tc.tile_pool(name="wop", bufs=1) as wop, \
                 tc.tile_pool(name="ps_o", bufs=3, space="PSUM") as ps_o, \
                 tc.tile_pool(name="op", bufs=3) as op:
                wo_sb = [wop.tile([128, D], F32R, name=f"wo{i}") for i in range(2)]
                for hp in range(2):
                    nc.sync.dma_start(
                        out=wo_sb[hp],
                        in_=wo_d[hp * 128:(hp + 1) * 128, :].bitcast(F32R))
                for qt in range(K // 128):
                    o_ps = ps_o.tile([128, D], F32, name="o_ps")
                    for hp in range(2):
                        nc.tensor.matmul(
                            o_ps[:], ac[hp][:, qt * 128:(qt + 1) * 128],
                            wo_sb[hp][:], start=(hp == 0), stop=(hp == 1))
                    o_sb = op.tile([128, D], F32, name="o_sb")
                    nc.vector.tensor_add(out=o_sb[:], in0=o_ps[:], in1=bo_bc[:])
                    nc.sync.dma_start(
                        out=rs_in[qt * 128:(qt + 1) * 128, :], in_=o_sb[:])
            nc.gpsimd.collective_compute(
                "ReduceScatter", mybir.AluOpType.add,
                replica_groups=[[0, 1], [2, 3], [4, 5], [6, 7]],
                ins=[rs_in[:]], outs=[rs_out[:]])

            # ---------------- phase 4: residual + LN1 + h1T ----------------
            h1_pool = stack.enter_context(tc.tile_pool(name="h1", bufs=1))
            h1t_pool = stack.enter_context(tc.tile_pool(name="h1t", bufs=1))
            h1 = [h1_pool.tile([128, D], F32, name=f"h1_{i}")
                  for i in range(OWN // 128)]
            h1T = [h1t_pool.tile([128, OWN], F32R, name=f"h1T{i}")
                   for i in range(4)]

            def layer_norm(dst, src_ps_or_sb, res_sb, g_bc, be_bc, pool,
                           extra_bc=None):
                """dst = g * norm(src + res [+ extra]) + be (src may be PSUM)."""
                pre = pool.tile([128, D], F32, name="ln_pre")
                nc.vector.tensor_add(out=pre[:], in0=src_ps_or_sb, in1=res_sb)
                if extra_bc is not None:
                    nc.vector.tensor_add(out=pre[:], in0=pre[:], in1=extra_bc[:])
                stats = pool.tile([128, 6], F32, name="ln_stats")
                nc.vector.bn_stats(out=stats[:], in_=pre[:])
                mv = pool.tile([128, 2], F32, name="ln_mv")
                nc.vector.bn_aggr(out=mv[:], in_=stats[:])
                rstd = pool.tile([128, 1], F32, name="ln_rstd")
                nc.scalar.activation(out=rstd[:], in_=mv[:, 1:2],
                                     func=AF.Sqrt, bias=eps_t[:])
                nc.vector.reciprocal(out=rstd[:], in_=rstd[:])
                nc.vector.tensor_scalar(
                    out=pre[:], in0=pre[:], scalar1=mv[:, 0:1],
                    scalar2=rstd[:], op0=mybir.AluOpType.subtract,
                    op1=mybir.AluOpType.mult)
                nc.vector.tensor_mul(out=pre[:], in0=pre[:], in1=g_bc[:])
                nc.vector.tensor_add(out=dst, in0=pre[:], in1=be_bc[:])

            with tc.tile_pool(name="lnp", bufs=4) as lnp, \
                 tc.tile_pool(name="ps_tr4", bufs=2, space="PSUM") as ps_tr, \
                 tc.tile_pool(name="xrp", bufs=3) as xrp:
                for qt in range(OWN // 128):
                    ored = lnp.tile([128, D], F32, name="ored")
                    nc.sync.dma_start(
                        out=ored, in_=rs_out[qt * 128:(qt + 1) * 128, :])
                    xr = xrp.tile([128, D], F32, name="xr")
                    nc.sync.dma_start(
                        out=xr, in_=xres_d[qt * 128:(qt + 1) * 128, :])
                    layer_norm(h1[qt][:], ored[:], xr[:], g1_bc, be1_bc, lnp)
                    for dc in range(4):
                        trp = ps_tr.tile([128, 128], F32, name="trp")
                        nc.tensor.transpose(
                            trp[:], h1[qt][:, dc * 128:(dc + 1) * 128],
                            ident[:])
                        nc.scalar.copy(
                            out=h1T[dc][:, qt * 128:(qt + 1) * 128],
                            in_=trp[:])

            # ---------------- phase 5: FFN + LN2 + out ----------------
            with tc.tile_pool(name="fw", bufs=1) as fw, \
                 tc.tile_pool(name="ps_f1", bufs=3, space="PSUM") as ps_f1, \
                 tc.tile_pool(name="ps_f2", bufs=1, space="PSUM") as ps_f2, \
                 tc.tile_pool(name="fap", bufs=3) as fap, \
                 tc.tile_pool(name="outp", bufs=3) as outp:
                w1_sb = [fw.tile([128, F], F32R, name=f"w1_{i}") for i in range(4)]
                for dc in range(4):
                    nc.sync.dma_start(
                        out=w1_sb[dc],
                        in_=w1_d[dc * 128:(dc + 1) * 128, :].bitcast(F32R))
                w2_sb = [fw.tile([128, D], F32R, name=f"w2_{i}")
                         for i in range(16)]
                for fc in range(16):
                    nc.sync.dma_start(
                        out=w2_sb[fc],
                        in_=w2_d[fc * 128:(fc + 1) * 128, :].bitcast(F32R))

                for qb2 in range(2):
                    ff2_ps = [ps_f2.tile([128, D], F32, name=f"ff2_{i}")
                              for i in range(4)]
                    for fc in range(16):
                        fp_ps = ps_f1.tile([128, 512], F32, name="fp_ps")
                        for dc in range(4):
                            nc.tensor.matmul(
                                fp_ps[:],
                                w1_sb[dc][:, fc * 128:(fc + 1) * 128],
                                h1T[dc][:, qb2 * 512:(qb2 + 1) * 512],
                                start=(dc == 0), stop=(dc == 3))
                        fa = fap.tile([128, 512], F32R, name="fa")
                        nc.scalar.activation(out=fa[:], in_=fp_ps[:],
                                             func=AF.Relu,
                                             bias=b1_col[:, fc:fc + 1])
                        for qt2 in range(4):
                            nc.tensor.matmul(
                                ff2_ps[qt2][:],
                                fa[:, qt2 * 128:(qt2 + 1) * 128],
                                w2_sb[fc][:], start=(fc == 0), stop=(fc == 15))
                    for qt2 in range(4):
                        qt = qb2 * 4 + qt2
                        out_sb = outp.tile([128, D], F32, name="out_sb")
                        layer_norm(out_sb[:], ff2_ps[qt2][:], h1[qt][:],
                                   g2_bc, be2_bc, outp, extra_bc=b2_bc)
                        nc.sync.dma_start(
                            out=out_d[qt * 128:(qt + 1) * 128, :],
                            in_=out_sb[:])

    nc.compile()
    return nc


def _get_nc():
    if "nc" not in _CACHE:
        _CACHE["nc"] = _build()
    return _CACHE["nc"]


def kernel(x, Wq, bq, Wk, bk, Wv, bv, Wo, bo, W1, b1, W2, b2, g1, be1, g2,
           be2, mask=None, **_unused):
    nc = _get_nc()
    in_maps = _make_in_maps(x, Wq, bq, Wk, bk, Wv, bv, Wo, bo, W1, b1, W2, b2,
                            g1, be1, g2, be2)

    res = bass_utils.run_bass_kernel_spmd(
        nc, in_maps, core_ids=list(range(N_CORES)))

    y = np.empty((N, K, D), np.float32)
    for c in range(N_CORES):
        n, s = divmod(c, 2)
        y[n, OWN * s:OWN * s + OWN] = res.results[c]["out"]
    return y


def _make_in_maps(x, Wq, bq, Wk, bk, Wv, bv, Wo, bo, W1, b1, W2, b2, g1, be1,
                  g2, be2):
    x = np.ascontiguousarray(np.asarray(x, dtype=np.float32))
    Wq, Wk, Wv = (np.asarray(w, np.float32) for w in (Wq, Wk, Wv))
    in_maps = []
    for c in range(N_CORES):
        n, s = divmod(c, 2)
        hsel = slice(HH * s, HH * s + HH)
        in_maps.append({
            "xn": x[n],
            "xres": x[n, OWN * s:OWN * s + OWN],
            "wq": np.ascontiguousarray(Wq[hsel].transpose(1, 0, 2).reshape(D, E)),
            "wk": np.ascontiguousarray(Wk[hsel].transpose(1, 0, 2).reshape(D, E)),
            "wv": np.ascontiguousarray(Wv[hsel].transpose(1, 0, 2).reshape(D, E)),
            "bq2": np.ascontiguousarray(np.asarray(bq, np.float32)[hsel]).reshape(1, E),
            "bk2": np.ascontiguousarray(np.asarray(bk, np.float32)[hsel]).reshape(1, E),
            "bv2": np.ascontiguousarray(np.asarray(bv, np.float32)[hsel]).reshape(1, E),
            "wo": np.ascontiguousarray(np.asarray(Wo, np.float32)[E * s:E * s + E]),
            "bo2": (np.asarray(bo, np.float32) * 0.5).reshape(1, D),
            "w1": np.asarray(W1, np.float32),
            "b12": np.asarray(b1, np.float32).reshape(1, F),
            "w2": np.asarray(W2, np.float32),
            "b22": np.asarray(b2, np.float32).reshape(1, D),
            "g1": np.asarray(g1, np.float32).reshape(1, D),
            "be1": np.asarray(be1, np.float32).reshape(1, D),
            "g2": np.asarray(g2, np.float32).reshape(1, D),
            "be2": np.asarray(be2, np.float32).reshape(1, D),
        })
    return in_maps


def kernel_timed(x, Wq, bq, Wk, bk, Wv, bv, Wo, bo, W1, b1, W2, b2, g1, be1,
                 g2, be2, mask=None, **_unused):
    """Run with NTFF tracing; returns BassKernelResults (exec_time_ns etc)."""
    nc = _get_nc()
    in_maps = _make_in_maps(x, Wq, bq, Wk, bk, Wv, bv, Wo, bo, W1, b1, W2, b2,
                            g1, be1, g2, be2)
    return bass_utils.run_bass_kernel_spmd(
        nc, in_maps, core_ids=list(range(N_CORES)), trace=True,
        trace_cores=list(range(N_CORES)))
